# revision 4
# baseline (speedup 1.0000x reference)
"""Trainium2 Bass kernel for nn_Bidir_Attention (top-k masked bidirectional
cross-attention) — v2.

Data-parallel over batch: each of the 8 NeuronCores processes one batch
element end-to-end. All matmuls run at 1 cycle/row using fp16 operands:
the selection-critical GEMMs (QK projection, scores) use a 3-term fp16
hi/lo decomposition with the lo part scaled by 2^11 (keeps it in the
fp16 normal range at full 11-bit precision): x ~ xh + xl'/2048 with
xl' = fp16((x - xh)*2048). The hi*hi product accumulates in one PSUM
bank, the two cross products (both carrying a 2^11 factor) in a second;
they merge as a + b/2048. Measures fp32-grade on HW (8.5e-7 absmax
score error vs fp32's 9.0e-7). V and the masked A@V use single fp16.

Self-contained: hardcodes B=8, N=2048, D=1024, topk=16.
"""

import sys

import numpy as np

for _p in ("/opt/trn_rl_repo", "/root/.axon_site/_ro/trn_rl_repo"):
    if _p not in sys.path:
        sys.path.append(_p)

import concourse.bacc as bacc
import concourse.mybir as mybir
from concourse.tile import TileContext
from concourse.masks import make_identity
from concourse.bass_utils import run_bass_kernel_spmd

B = 8
N = 2048
D = 1024
NT = N // 128          # 16 row tiles
DT = D // 128          # 8 contraction tiles
TOPK = 16
SCALE = float(1.0 / np.sqrt(D))
NEG = -1e30
F32 = mybir.dt.float32
F16 = mybir.dt.float16
LOSC = 2048.0          # lo-part scale (2^11) keeping fp16 lo in normal range
ILOSC = float(1.0 / LOSC)


def _phase_a(nc, pools, x_dram, ident32, wqk_h, wqk_l, wv_h,
             qh_dram, ql_dram, kh_dram, kl_dram, v_dram):
    """QKV for one feature. Q^T/K^T written as fp16 hi/lo pairs [D,N],
    V written fp16 [N,D]. Q pre-scaled by 1/sqrt(D)."""
    sb, ps = pools
    for j in range(4):                      # supertiles of 512 rows
        xs = []
        for nsub in range(4):
            x = sb.tile([128, D], F32, tag=f"x{nsub}", bufs=2)
            nc.sync.dma_start(
                out=x[:], in_=x_dram.ap()[j * 512 + nsub * 128: j * 512 + (nsub + 1) * 128, :])
            xs.append(x)
        xh = sb.tile([128, DT, 512], F16, tag="xh", bufs=2)
        xl = sb.tile([128, DT, 512], F16, tag="xl", bufs=2)
        for nsub in range(4):
            for di in range(DT):
                tp = ps.tile([128, 128], F32, tag="tp", bufs=2)
                nc.tensor.transpose(tp[:], xs[nsub][:, di * 128:(di + 1) * 128], ident32[:])
                sl = (di, slice(nsub * 128, (nsub + 1) * 128))
                nc.scalar.copy(xh[:, sl[0], sl[1]], tp[:])
                xtmp = sb.tile([128, 128], F32, tag="xtmp", bufs=3)
                nc.vector.tensor_sub(xtmp[:], tp[:], xh[:, sl[0], sl[1]])
                nc.scalar.mul(xl[:, sl[0], sl[1]], xtmp[:], LOSC)
        # Q^T and K^T: hi*hi into bank a, scaled cross terms into bank b
        for t in range(16):
            qk_a = ps.tile([128, 512], F32, tag="qk_a", bufs=2)
            qk_b = ps.tile([128, 512], F32, tag="qk_b", bufs=2)
            tsl = slice(t * 128, (t + 1) * 128)
            for di in range(DT):
                nc.tensor.matmul(qk_a[:], wqk_h[di][:, tsl], xh[:, di, :],
                                 start=(di == 0), stop=(di == DT - 1))
                nc.tensor.matmul(qk_b[:], wqk_h[di][:, tsl], xl[:, di, :],
                                 start=(di == 0), stop=False)
            for di in range(DT):
                nc.tensor.matmul(qk_b[:], wqk_l[di][:, tsl], xh[:, di, :],
                                 start=False, stop=(di == DT - 1))
            sc = SCALE if t < 8 else 1.0
            h_dst = qh_dram if t < 8 else kh_dram
            l_dst = ql_dram if t < 8 else kl_dram
            r0 = (t % 8) * 128
            # full-precision merged value: fa = a*sc ; ssum = fa + b*(sc/2048)
            fa = sb.tile([128, 512], F32, tag="fa", bufs=2)
            nc.scalar.activation(fa[:], qk_a[:], mybir.ActivationFunctionType.Copy,
                                 scale=sc)
            ssum = sb.tile([128, 512], F32, tag="ssum", bufs=2)
            nc.vector.scalar_tensor_tensor(out=ssum[:], in0=qk_b[:],
                                           scalar=float(sc / LOSC), in1=fa[:],
                                           op0=mybir.AluOpType.mult,
                                           op1=mybir.AluOpType.add)
            hsb = sb.tile([128, 512], F16, tag="hsb", bufs=3)
            lsb = sb.tile([128, 512], F16, tag="lsb", bufs=3)
            nc.scalar.copy(hsb[:], ssum[:])
            qtmp = sb.tile([128, 512], F32, tag="qtmp", bufs=2)
            nc.vector.tensor_sub(qtmp[:], ssum[:], hsb[:])
            nc.scalar.mul(lsb[:], qtmp[:], LOSC)
            nc.gpsimd.dma_start(
                out=h_dst.ap()[r0:r0 + 128, j * 512:(j + 1) * 512], in_=hsb[:])
            nc.gpsimd.dma_start(
                out=l_dst.ap()[r0:r0 + 128, j * 512:(j + 1) * 512], in_=lsb[:])
        # V: natural layout [n 128-tile, dout 512] pieces, single fp16
        for nsub in range(4):
            nsl = slice(nsub * 128, (nsub + 1) * 128)
            v_ps0 = ps.tile([128, 512], F32, tag="v_ps0", bufs=1)
            v_ps1 = ps.tile([128, 512], F32, tag="v_ps1", bufs=1)
            for di in range(DT):
                st = (di == 0)
                sp = (di == DT - 1)
                nc.tensor.matmul(v_ps0[:], xh[:, di, nsl], wv_h[di][:, 0:512],
                                 start=st, stop=sp)
                nc.tensor.matmul(v_ps1[:], xh[:, di, nsl], wv_h[di][:, 512:1024],
                                 start=st, stop=sp)
            for c, vp in ((0, v_ps0), (1, v_ps1)):
                vsb = sb.tile([128, 512], F16, tag="vsb", bufs=3)
                nc.scalar.copy(vsb[:], vp[:])
                nc.gpsimd.dma_start(
                    out=v_dram.ap()[j * 512 + nsub * 128: j * 512 + (nsub + 1) * 128,
                                    c * 512:(c + 1) * 512],
                    in_=vsb[:])


def _phase_b(nc, pools, ident16, qh_dram, ql_dram, kh_dram, kl_dram, v_dram, out_dram,
             dtag):
    """One attention direction: S (pre-scaled) via 3-term fp16 hi/lo,
    softmax stats, exact top-16 via 2x(max8+match_replace), fp16 AV,
    1/Z renorm. Resident loads spread over three DMA queues; V tiles get
    per-direction tags so direction 2's V prefetches during direction 1."""
    sbr, sb, ps = pools
    qs = (nc.sync, nc.scalar, nc.gpsimd)
    kh = []
    kl = []
    for di in range(DT):
        th = sbr.tile([128, N], F16, tag=f"kh{di}", name=f"kh{di}")
        tl = sbr.tile([128, N], F16, tag=f"kl{di}", name=f"kl{di}")
        qs[di % 3].dma_start(out=th[:], in_=kh_dram.ap()[di * 128:(di + 1) * 128, :])
        qs[(di + 1) % 3].dma_start(out=tl[:], in_=kl_dram.ap()[di * 128:(di + 1) * 128, :])
        kh.append(th)
        kl.append(tl)
    vres = []
    for nt in range(NT):
        t = sbr.tile([128, D], F16, tag=f"v{nt}d{dtag}", name=f"v{nt}d{dtag}")
        qs[nt % 3].dma_start(out=t[:], in_=v_dram.ap()[nt * 128:(nt + 1) * 128, :])
        vres.append(t)

    for qi in range(NT):
        qh_t = sb.tile([128, DT, 128], F16, tag="qh_t", bufs=2)
        ql_t = sb.tile([128, DT, 128], F16, tag="ql_t", bufs=2)
        for di in range(DT):
            nc.sync.dma_start(
                out=qh_t[:, di, :],
                in_=qh_dram.ap()[di * 128:(di + 1) * 128, qi * 128:(qi + 1) * 128])
            nc.sync.dma_start(
                out=ql_t[:, di, :],
                in_=ql_dram.ap()[di * 128:(di + 1) * 128, qi * 128:(qi + 1) * 128])
        ssb = sb.tile([128, N], F32, tag="ssb", bufs=2)
        # chunk-paired: each loaded q weight serves 4 matmuls (2 chunks x kh/kl)
        for half in range(2):
            s_a0 = ps.tile([128, 512], F32, tag="s_a0", bufs=1)
            s_a1 = ps.tile([128, 512], F32, tag="s_a1", bufs=1)
            s_b0 = ps.tile([128, 512], F32, tag="s_b0", bufs=1)
            s_b1 = ps.tile([128, 512], F32, tag="s_b1", bufs=1)
            c0 = slice(half * 1024, half * 1024 + 512)
            c1 = slice(half * 1024 + 512, half * 1024 + 1024)
            for di in range(DT):
                st = (di == 0)
                sp = (di == DT - 1)
                nc.tensor.matmul(s_a0[:], qh_t[:, di, :], kh[di][:, c0], start=st, stop=sp)
                nc.tensor.matmul(s_a1[:], qh_t[:, di, :], kh[di][:, c1], start=st, stop=sp)
                nc.tensor.matmul(s_b0[:], qh_t[:, di, :], kl[di][:, c0], start=st, stop=False)
                nc.tensor.matmul(s_b1[:], qh_t[:, di, :], kl[di][:, c1], start=st, stop=False)
            for di in range(DT):
                sp = (di == DT - 1)
                nc.tensor.matmul(s_b0[:], ql_t[:, di, :], kh[di][:, c0], start=False, stop=sp)
                nc.tensor.matmul(s_b1[:], ql_t[:, di, :], kh[di][:, c1], start=False, stop=sp)
            for c, (sa, sbk) in ((c0, (s_a0, s_b0)), (c1, (s_a1, s_b1))):
                nc.scalar.copy(ssb[:, c], sa[:])
                nc.vector.scalar_tensor_tensor(out=ssb[:, c], in0=sbk[:],
                                               scalar=ILOSC, in1=ssb[:, c],
                                               op0=mybir.AluOpType.mult,
                                               op1=mybir.AluOpType.add)

        m0 = sb.tile([128, 8], F32, tag="m0")
        nc.vector.max(out=m0[:], in_=ssb[:])
        nm = sb.tile([128, 1], F32, tag="nm")
        nc.vector.tensor_scalar_mul(nm[:], m0[:, 0:1], -1.0)
        p16 = sb.tile([128, N], F16, tag="p16", bufs=2)
        z = sb.tile([128, 1], F32, tag="z")
        nc.scalar.activation(p16[:], ssb[:], mybir.ActivationFunctionType.Exp,
                             bias=nm[:], scale=1.0, accum_out=z[:])
        iz = sb.tile([128, 1], F32, tag="iz")
        nc.vector.reciprocal(iz[:], z[:])
        # exact top-16: two rounds of max8 + match_replace on ssb
        nc.vector.match_replace(out=ssb[:], in_to_replace=m0[:], in_values=ssb[:],
                                imm_value=NEG)
        m8 = sb.tile([128, 8], F32, tag="m8")
        nc.vector.max(out=m8[:], in_=ssb[:])
        nc.vector.match_replace(out=ssb[:], in_to_replace=m8[:], in_values=ssb[:],
                                imm_value=NEG)
        # masked probs: pm = (ssb == NEG) * p16   (fp16)
        pm = sb.tile([128, N], F16, tag="pm", bufs=2)
        nc.vector.scalar_tensor_tensor(out=pm[:], in0=ssb[:], scalar=NEG, in1=p16[:],
                                       op0=mybir.AluOpType.is_equal,
                                       op1=mybir.AluOpType.mult)
        # transpose A tiles (fp16) for the AV matmul
        ah = sb.tile([128, NT, 128], F16, tag="ah", bufs=2)
        for kt_i in range(NT):
            tp2 = ps.tile([128, 128], F16, tag="tp2", bufs=2)
            nc.tensor.transpose(tp2[:], pm[:, kt_i * 128:(kt_i + 1) * 128], ident16[:])
            if kt_i % 2:
                nc.vector.tensor_copy(ah[:, kt_i, :], tp2[:])
            else:
                nc.scalar.copy(ah[:, kt_i, :], tp2[:])
        osb = sb.tile([128, D], F32, tag="osb", bufs=2)
        o_ps0 = ps.tile([128, 512], F32, tag="o_ps0", bufs=1)
        o_ps1 = ps.tile([128, 512], F32, tag="o_ps1", bufs=1)
        for kt_i in range(NT):
            st = (kt_i == 0)
            sp = (kt_i == NT - 1)
            nc.tensor.matmul(o_ps0[:], ah[:, kt_i, :], vres[kt_i][:, 0:512],
                             start=st, stop=sp)
            nc.tensor.matmul(o_ps1[:], ah[:, kt_i, :], vres[kt_i][:, 512:1024],
                             start=st, stop=sp)
        nc.scalar.activation(osb[:, 0:512], o_ps0[:],
                             mybir.ActivationFunctionType.Copy, scale=iz[:])
        nc.scalar.activation(osb[:, 512:1024], o_ps1[:],
                             mybir.ActivationFunctionType.Copy, scale=iz[:])
        nc.gpsimd.dma_start(out=out_dram.ap()[qi * 128:(qi + 1) * 128, :], in_=osb[:])


def build(repeat=1):
    nc = bacc.Bacc()
    f1 = nc.declare_dram_parameter("feature1", [N, D], F32, isOutput=False)
    f2 = nc.declare_dram_parameter("feature2", [N, D], F32, isOutput=False)
    w = nc.declare_dram_parameter("w_qkv", [D, 3 * D], F32, isOutput=False)
    out1 = nc.declare_dram_parameter("out1", [N, D], F32, isOutput=True)
    out2 = nc.declare_dram_parameter("out2", [N, D], F32, isOutput=True)

    scr = {}
    for feat in (1, 2):
        for nm in ("qh", "ql", "kh", "kl"):
            scr[f"{nm}{feat}"] = nc.dram_tensor(f"{nm}{feat}", [D, N], F16)
        scr[f"v{feat}"] = nc.dram_tensor(f"v{feat}", [N, D], F16)

    with TileContext(nc) as tc:
        with tc.tile_pool(name="const", bufs=1) as constp:
            ident32 = constp.tile([128, 128], F32, tag="id32")
            make_identity(nc, ident32[:])
            ident16 = constp.tile([128, 128], F16, tag="id16")
            make_identity(nc, ident16[:])

            for _rep in range(repeat):
                with (
                    tc.tile_pool(name="wpool", bufs=1) as wp,
                    tc.tile_pool(name="apool", bufs=1) as asb,
                    tc.tile_pool(name="apsum", bufs=1, space="PSUM") as aps,
                ):
                    wqk_h, wqk_l, wv_h = [], [], []
                    for di in range(DT):
                        wst = asb.tile([128, 3 * D], F32, tag="wst", bufs=2)
                        (nc.sync if di % 2 == 0 else nc.scalar).dma_start(
                            out=wst[:], in_=w.ap()[di * 128:(di + 1) * 128, :])
                        wh = wp.tile([128, 2 * D], F16, tag=f"wqh{di}", name=f"wqh{di}")
                        wl = wp.tile([128, 2 * D], F16, tag=f"wql{di}", name=f"wql{di}")
                        nc.vector.tensor_copy(wh[:], wst[:, :2 * D])
                        wtmp = asb.tile([128, 2 * D], F32, tag="wtmp", bufs=2)
                        nc.vector.tensor_sub(wtmp[:], wst[:, :2 * D], wh[:])
                        nc.scalar.mul(wl[:], wtmp[:], LOSC)
                        vh = wp.tile([128, D], F16, tag=f"wvh{di}", name=f"wvh{di}")
                        nc.scalar.copy(vh[:], wst[:, 2 * D:])
                        wqk_h.append(wh)
                        wqk_l.append(wl)
                        wv_h.append(vh)
                    _phase_a(nc, (asb, aps), f1, ident32, wqk_h, wqk_l, wv_h,
                             scr["qh1"], scr["ql1"], scr["kh1"], scr["kl1"], scr["v1"])
                    _phase_a(nc, (asb, aps), f2, ident32, wqk_h, wqk_l, wv_h,
                             scr["qh2"], scr["ql2"], scr["kh2"], scr["kl2"], scr["v2"])

                with (
                    tc.tile_pool(name="bpool", bufs=1) as bsb,
                    tc.tile_pool(name="bwork", bufs=1) as bwk,
                    tc.tile_pool(name="bpsum", bufs=1, space="PSUM") as bps,
                ):
                    _phase_b(nc, (bsb, bwk, bps), ident16,
                             scr["qh1"], scr["ql1"], scr["kh2"], scr["kl2"], scr["v2"], out1,
                             dtag=1)
                    _phase_b(nc, (bsb, bwk, bps), ident16,
                             scr["qh2"], scr["ql2"], scr["kh1"], scr["kl1"], scr["v1"], out2,
                             dtag=2)
    return nc


_NC_CACHE = None


def _get_nc():
    global _NC_CACHE
    if _NC_CACHE is None:
        _NC_CACHE = build()
        _NC_CACHE.finalize()
    return _NC_CACHE


def kernel(feature1, feature2, W_qkv, topk):
    assert int(topk) == TOPK, f"kernel hardcodes topk=16, got {topk}"
    f1 = np.ascontiguousarray(np.asarray(feature1), dtype=np.float32)
    f2 = np.ascontiguousarray(np.asarray(feature2), dtype=np.float32)
    w = np.ascontiguousarray(np.asarray(W_qkv), dtype=np.float32)
    assert f1.shape == (B, N, D) and f2.shape == (B, N, D) and w.shape == (D, 3 * D)

    nc = _get_nc()
    in_maps = [{"feature1": f1[b], "feature2": f2[b], "w_qkv": w} for b in range(B)]
    try:
        res = run_bass_kernel_spmd(nc, in_maps, list(range(B))).results
    except Exception:
        res = run_bass_kernel_spmd(nc, in_maps, list(range(B))).results
    o1 = np.stack([res[b]["out1"] for b in range(B)]).astype(np.float32)
    o2 = np.stack([res[b]["out2"] for b in range(B)]).astype(np.float32)
    return o1, o2


if __name__ == "__main__":
    f1 = np.load("/root/problem/cache/f1.npy")
    f2 = np.load("/root/problem/cache/f2.npy")
    w = np.load("/root/problem/cache/W.npy")
    o1, o2 = kernel(f1, f2, w, 16)
    r1 = np.load("/root/problem/cache/r1.npy")
    r2 = np.load("/root/problem/cache/r2.npy")
    for nm, o, r in (("2to1", o1, r1), ("1to2", o2, r2)):
        err = np.abs(o - r).max()
        rel = err / np.abs(r).max()
        print(f"{nm}: absmax_err={err:.3e} rel={rel:.3e}")


# revision 5
# speedup vs baseline: 1.4018x; 1.4018x over previous
"""Trainium2 Bass kernel for nn_Bidir_Attention (top-k masked bidirectional
cross-attention) — v2.

Data-parallel over batch: each of the 8 NeuronCores processes one batch
element end-to-end. All matmuls run at 1 cycle/row using fp16 operands:
the selection-critical GEMMs (QK projection, scores) use a 3-term fp16
hi/lo decomposition with the lo part scaled by 2^11 (keeps it in the
fp16 normal range at full 11-bit precision): x ~ xh + xl'/2048 with
xl' = fp16((x - xh)*2048). The hi*hi product accumulates in one PSUM
bank, the two cross products (both carrying a 2^11 factor) in a second;
they merge as a + b/2048. Measures fp32-grade on HW (8.5e-7 absmax
score error vs fp32's 9.0e-7). V and the masked A@V use single fp16.

Self-contained: hardcodes B=8, N=2048, D=1024, topk=16.
"""

import sys

import numpy as np

for _p in ("/opt/trn_rl_repo", "/root/.axon_site/_ro/trn_rl_repo"):
    if _p not in sys.path:
        sys.path.append(_p)

import concourse.bacc as bacc
import concourse.mybir as mybir
from concourse.tile import TileContext
from concourse.masks import make_identity
from concourse.bass_utils import run_bass_kernel_spmd

B = 8
N = 2048
D = 1024
NT = N // 128          # 16 row tiles
DT = D // 128          # 8 contraction tiles
TOPK = 16
SCALE = float(1.0 / np.sqrt(D))
NEG = -1e30
F32 = mybir.dt.float32
F16 = mybir.dt.float16
LOSC = 2048.0          # lo-part scale (2^11) keeping fp16 lo in normal range
ILOSC = float(1.0 / LOSC)


def _phase_a(nc, pools, x_dram, ident32, wqk_h, wqk_l, wv_h,
             qh_dram, ql_dram, kh_dram, kl_dram, v_dram, first=False):
    """QKV for one feature. Q^T/K^T written as fp16 hi/lo pairs [D,N],
    V written fp16 [N,D]. Q pre-scaled by 1/sqrt(D)."""
    sb, ps = pools
    for j in range(4):                      # supertiles of 512 rows
        xs = []
        for nsub in range(4):
            x = sb.tile([128, D], F32, tag=f"x{nsub}", bufs=2)
            xq = nc.gpsimd if (first and j == 0) else nc.sync
            xq.dma_start(
                out=x[:], in_=x_dram.ap()[j * 512 + nsub * 128: j * 512 + (nsub + 1) * 128, :])
            xs.append(x)
        xh = sb.tile([128, DT, 512], F16, tag="xh", bufs=2)
        xl = sb.tile([128, DT, 512], F16, tag="xl", bufs=2)
        for nsub in range(4):
            for di in range(DT):
                tp = ps.tile([128, 128], F32, tag="tp", bufs=2)
                nc.tensor.transpose(tp[:], xs[nsub][:, di * 128:(di + 1) * 128], ident32[:])
                sl = (di, slice(nsub * 128, (nsub + 1) * 128))
                nc.scalar.copy(xh[:, sl[0], sl[1]], tp[:])
                xtmp = sb.tile([128, 128], F32, tag="xtmp", bufs=3)
                nc.vector.tensor_sub(xtmp[:], tp[:], xh[:, sl[0], sl[1]])
                nc.scalar.mul(xl[:, sl[0], sl[1]], xtmp[:], LOSC)
        # Q^T and K^T: hi*hi into bank a, scaled cross terms into bank b
        for t in range(16):
            qk_a = ps.tile([128, 512], F32, tag="qk_a", bufs=2)
            qk_b = ps.tile([128, 512], F32, tag="qk_b", bufs=2)
            tsl = slice(t * 128, (t + 1) * 128)
            for di in range(DT):
                nc.tensor.matmul(qk_a[:], wqk_h[di][:, tsl], xh[:, di, :],
                                 start=(di == 0), stop=(di == DT - 1))
                nc.tensor.matmul(qk_b[:], wqk_h[di][:, tsl], xl[:, di, :],
                                 start=(di == 0), stop=False)
            for di in range(DT):
                nc.tensor.matmul(qk_b[:], wqk_l[di][:, tsl], xh[:, di, :],
                                 start=False, stop=(di == DT - 1))
            sc = SCALE if t < 8 else 1.0
            h_dst = qh_dram if t < 8 else kh_dram
            l_dst = ql_dram if t < 8 else kl_dram
            r0 = (t % 8) * 128
            # full-precision merged value: fa = a*sc ; ssum = fa + b*(sc/2048)
            fa = sb.tile([128, 512], F32, tag="fa", bufs=2)
            nc.scalar.activation(fa[:], qk_a[:], mybir.ActivationFunctionType.Copy,
                                 scale=sc)
            ssum = sb.tile([128, 512], F32, tag="ssum", bufs=2)
            nc.vector.scalar_tensor_tensor(out=ssum[:], in0=qk_b[:],
                                           scalar=float(sc / LOSC), in1=fa[:],
                                           op0=mybir.AluOpType.mult,
                                           op1=mybir.AluOpType.add)
            hsb = sb.tile([128, 512], F16, tag="hsb", bufs=3)
            lsb = sb.tile([128, 512], F16, tag="lsb", bufs=3)
            nc.scalar.copy(hsb[:], ssum[:])
            qtmp = sb.tile([128, 512], F32, tag="qtmp", bufs=2)
            nc.vector.tensor_sub(qtmp[:], ssum[:], hsb[:])
            nc.scalar.mul(lsb[:], qtmp[:], LOSC)
            nc.gpsimd.dma_start(
                out=h_dst.ap()[r0:r0 + 128, j * 512:(j + 1) * 512], in_=hsb[:])
            nc.gpsimd.dma_start(
                out=l_dst.ap()[r0:r0 + 128, j * 512:(j + 1) * 512], in_=lsb[:])
        # V: natural layout [n 128-tile, dout 512] pieces, single fp16
        for nsub in range(4):
            nsl = slice(nsub * 128, (nsub + 1) * 128)
            v_ps0 = ps.tile([128, 512], F32, tag="v_ps0", bufs=1)
            v_ps1 = ps.tile([128, 512], F32, tag="v_ps1", bufs=1)
            for di in range(DT):
                st = (di == 0)
                sp = (di == DT - 1)
                nc.tensor.matmul(v_ps0[:], xh[:, di, nsl], wv_h[di][:, 0:512],
                                 start=st, stop=sp)
                nc.tensor.matmul(v_ps1[:], xh[:, di, nsl], wv_h[di][:, 512:1024],
                                 start=st, stop=sp)
            for c, vp in ((0, v_ps0), (1, v_ps1)):
                vsb = sb.tile([128, 512], F16, tag="vsb", bufs=3)
                nc.scalar.copy(vsb[:], vp[:])
                nc.gpsimd.dma_start(
                    out=v_dram.ap()[j * 512 + nsub * 128: j * 512 + (nsub + 1) * 128,
                                    c * 512:(c + 1) * 512],
                    in_=vsb[:])


def _phase_b(nc, pools, ident16, qh_dram, ql_dram, kh_dram, kl_dram, v_dram, out_dram,
             dtag):
    """One attention direction: S (pre-scaled) via 3-term fp16 hi/lo,
    softmax stats, exact top-16 via 2x(max8+match_replace), fp16 AV,
    1/Z renorm. Resident loads spread over three DMA queues; V tiles get
    per-direction tags so direction 2's V prefetches during direction 1."""
    sbr, sb, ps = pools
    qs = (nc.sync, nc.scalar, nc.gpsimd)
    kh = []
    kl = []
    for di in range(DT):
        th = sbr.tile([128, N], F16, tag=f"kh{di}", name=f"kh{di}")
        tl = sbr.tile([128, N], F16, tag=f"kl{di}", name=f"kl{di}")
        qs[di % 3].dma_start(out=th[:], in_=kh_dram.ap()[di * 128:(di + 1) * 128, :])
        qs[(di + 1) % 3].dma_start(out=tl[:], in_=kl_dram.ap()[di * 128:(di + 1) * 128, :])
        kh.append(th)
        kl.append(tl)
    vres = []
    for nt in range(NT):
        t = sbr.tile([128, D], F16, tag=f"v{nt}d{dtag}", name=f"v{nt}d{dtag}")
        qs[nt % 3].dma_start(out=t[:], in_=v_dram.ap()[nt * 128:(nt + 1) * 128, :])
        vres.append(t)

    for qi in range(NT):
        qh_t = sb.tile([128, DT, 128], F16, tag="qh_t", bufs=2)
        ql_t = sb.tile([128, DT, 128], F16, tag="ql_t", bufs=2)
        for di in range(DT):
            nc.sync.dma_start(
                out=qh_t[:, di, :],
                in_=qh_dram.ap()[di * 128:(di + 1) * 128, qi * 128:(qi + 1) * 128])
            nc.sync.dma_start(
                out=ql_t[:, di, :],
                in_=ql_dram.ap()[di * 128:(di + 1) * 128, qi * 128:(qi + 1) * 128])
        ssb = sb.tile([128, N], F32, tag="ssb", bufs=2)
        # chunk-paired: each loaded q weight serves 4 matmuls (2 chunks x kh/kl)
        for half in range(2):
            s_a0 = ps.tile([128, 512], F32, tag="s_a0", bufs=1)
            s_a1 = ps.tile([128, 512], F32, tag="s_a1", bufs=1)
            s_b0 = ps.tile([128, 512], F32, tag="s_b0", bufs=1)
            s_b1 = ps.tile([128, 512], F32, tag="s_b1", bufs=1)
            c0 = slice(half * 1024, half * 1024 + 512)
            c1 = slice(half * 1024 + 512, half * 1024 + 1024)
            for di in range(DT):
                st = (di == 0)
                sp = (di == DT - 1)
                nc.tensor.matmul(s_a0[:], qh_t[:, di, :], kh[di][:, c0], start=st, stop=sp)
                nc.tensor.matmul(s_a1[:], qh_t[:, di, :], kh[di][:, c1], start=st, stop=sp)
                nc.tensor.matmul(s_b0[:], qh_t[:, di, :], kl[di][:, c0], start=st, stop=False)
                nc.tensor.matmul(s_b1[:], qh_t[:, di, :], kl[di][:, c1], start=st, stop=False)
            for di in range(DT):
                sp = (di == DT - 1)
                nc.tensor.matmul(s_b0[:], ql_t[:, di, :], kh[di][:, c0], start=False, stop=sp)
                nc.tensor.matmul(s_b1[:], ql_t[:, di, :], kh[di][:, c1], start=False, stop=sp)
            for c, (sa, sbk) in ((c0, (s_a0, s_b0)), (c1, (s_a1, s_b1))):
                nc.scalar.copy(ssb[:, c], sa[:])
                nc.vector.scalar_tensor_tensor(out=ssb[:, c], in0=sbk[:],
                                               scalar=ILOSC, in1=ssb[:, c],
                                               op0=mybir.AluOpType.mult,
                                               op1=mybir.AluOpType.add)

        m0 = sb.tile([128, 8], F32, tag="m0")
        nc.vector.max(out=m0[:], in_=ssb[:])
        nm = sb.tile([128, 1], F32, tag="nm")
        nc.vector.tensor_scalar_mul(nm[:], m0[:, 0:1], -1.0)
        p16 = sb.tile([128, N], F16, tag="p16", bufs=2)
        z = sb.tile([128, 1], F32, tag="z")
        nc.scalar.activation(p16[:], ssb[:], mybir.ActivationFunctionType.Exp,
                             bias=nm[:], scale=1.0, accum_out=z[:])
        iz = sb.tile([128, 1], F32, tag="iz")
        nc.vector.reciprocal(iz[:], z[:])
        # exact top-16: two rounds of max8 + match_replace on ssb
        nc.vector.match_replace(out=ssb[:], in_to_replace=m0[:], in_values=ssb[:],
                                imm_value=NEG)
        m8 = sb.tile([128, 8], F32, tag="m8")
        nc.vector.max(out=m8[:], in_=ssb[:])
        nc.vector.match_replace(out=ssb[:], in_to_replace=m8[:], in_values=ssb[:],
                                imm_value=NEG)
        # masked probs: pm = (ssb == NEG) * p16   (fp16)
        pm = sb.tile([128, N], F16, tag="pm", bufs=2)
        nc.vector.scalar_tensor_tensor(out=pm[:], in0=ssb[:], scalar=NEG, in1=p16[:],
                                       op0=mybir.AluOpType.is_equal,
                                       op1=mybir.AluOpType.mult)
        # transpose A tiles (fp16) for the AV matmul
        ah = sb.tile([128, NT, 128], F16, tag="ah", bufs=2)
        for kt_i in range(NT):
            tp2 = ps.tile([128, 128], F16, tag="tp2", bufs=2)
            nc.tensor.transpose(tp2[:], pm[:, kt_i * 128:(kt_i + 1) * 128], ident16[:])
            if kt_i % 2:
                nc.vector.tensor_copy(ah[:, kt_i, :], tp2[:])
            else:
                nc.scalar.copy(ah[:, kt_i, :], tp2[:])
        osb = sb.tile([128, D], F32, tag="osb", bufs=2)
        o_ps0 = ps.tile([128, 512], F32, tag="o_ps0", bufs=1)
        o_ps1 = ps.tile([128, 512], F32, tag="o_ps1", bufs=1)
        for kt_i in range(NT):
            st = (kt_i == 0)
            sp = (kt_i == NT - 1)
            nc.tensor.matmul(o_ps0[:], ah[:, kt_i, :], vres[kt_i][:, 0:512],
                             start=st, stop=sp)
            nc.tensor.matmul(o_ps1[:], ah[:, kt_i, :], vres[kt_i][:, 512:1024],
                             start=st, stop=sp)
        nc.scalar.activation(osb[:, 0:512], o_ps0[:],
                             mybir.ActivationFunctionType.Copy, scale=iz[:])
        nc.scalar.activation(osb[:, 512:1024], o_ps1[:],
                             mybir.ActivationFunctionType.Copy, scale=iz[:])
        nc.gpsimd.dma_start(out=out_dram.ap()[qi * 128:(qi + 1) * 128, :], in_=osb[:])


def build(repeat=1):
    nc = bacc.Bacc()
    f1 = nc.declare_dram_parameter("feature1", [N, D], F32, isOutput=False)
    f2 = nc.declare_dram_parameter("feature2", [N, D], F32, isOutput=False)
    w = nc.declare_dram_parameter("w_qkv", [D, 3 * D], F32, isOutput=False)
    out1 = nc.declare_dram_parameter("out1", [N, D], F32, isOutput=True)
    out2 = nc.declare_dram_parameter("out2", [N, D], F32, isOutput=True)

    scr = {}
    for feat in (1, 2):
        for nm in ("qh", "ql", "kh", "kl"):
            scr[f"{nm}{feat}"] = nc.dram_tensor(f"{nm}{feat}", [D, N], F16)
        scr[f"v{feat}"] = nc.dram_tensor(f"v{feat}", [N, D], F16)

    with TileContext(nc) as tc:
        with tc.tile_pool(name="const", bufs=1) as constp:
            ident32 = constp.tile([128, 128], F32, tag="id32")
            make_identity(nc, ident32[:])
            ident16 = constp.tile([128, 128], F16, tag="id16")
            make_identity(nc, ident16[:])

            for _rep in range(repeat):
                with (
                    tc.tile_pool(name="wpool", bufs=1) as wp,
                    tc.tile_pool(name="apool", bufs=1) as asb,
                    tc.tile_pool(name="apsum", bufs=1, space="PSUM") as aps,
                ):
                    wqk_h, wqk_l, wv_h = [], [], []
                    for di in range(DT):
                        wst = asb.tile([128, 3 * D], F32, tag="wst", bufs=2)
                        (nc.sync if di % 2 == 0 else nc.scalar).dma_start(
                            out=wst[:, :2 * D],
                            in_=w.ap()[di * 128:(di + 1) * 128, :2 * D])
                        nc.gpsimd.dma_start(
                            out=wst[:, 2 * D:],
                            in_=w.ap()[di * 128:(di + 1) * 128, 2 * D:])
                        wh = wp.tile([128, 2 * D], F16, tag=f"wqh{di}", name=f"wqh{di}")
                        wl = wp.tile([128, 2 * D], F16, tag=f"wql{di}", name=f"wql{di}")
                        nc.vector.tensor_copy(wh[:], wst[:, :2 * D])
                        wtmp = asb.tile([128, 2 * D], F32, tag="wtmp", bufs=2)
                        nc.vector.tensor_sub(wtmp[:], wst[:, :2 * D], wh[:])
                        nc.scalar.mul(wl[:], wtmp[:], LOSC)
                        vh = wp.tile([128, D], F16, tag=f"wvh{di}", name=f"wvh{di}")
                        nc.scalar.copy(vh[:], wst[:, 2 * D:])
                        wqk_h.append(wh)
                        wqk_l.append(wl)
                        wv_h.append(vh)
                    _phase_a(nc, (asb, aps), f1, ident32, wqk_h, wqk_l, wv_h,
                             scr["qh1"], scr["ql1"], scr["kh1"], scr["kl1"], scr["v1"],
                             first=True)
                    _phase_a(nc, (asb, aps), f2, ident32, wqk_h, wqk_l, wv_h,
                             scr["qh2"], scr["ql2"], scr["kh2"], scr["kl2"], scr["v2"])

                with (
                    tc.tile_pool(name="bpool", bufs=1) as bsb,
                    tc.tile_pool(name="bwork", bufs=1) as bwk,
                    tc.tile_pool(name="bpsum", bufs=1, space="PSUM") as bps,
                ):
                    _phase_b(nc, (bsb, bwk, bps), ident16,
                             scr["qh1"], scr["ql1"], scr["kh2"], scr["kl2"], scr["v2"], out1,
                             dtag=1)
                    _phase_b(nc, (bsb, bwk, bps), ident16,
                             scr["qh2"], scr["ql2"], scr["kh1"], scr["kl1"], scr["v1"], out2,
                             dtag=2)
    return nc


_NC_CACHE = None


def _get_nc():
    global _NC_CACHE
    if _NC_CACHE is None:
        _NC_CACHE = build()
        _NC_CACHE.finalize()
    return _NC_CACHE


def kernel(feature1, feature2, W_qkv, topk):
    assert int(topk) == TOPK, f"kernel hardcodes topk=16, got {topk}"
    f1 = np.ascontiguousarray(np.asarray(feature1), dtype=np.float32)
    f2 = np.ascontiguousarray(np.asarray(feature2), dtype=np.float32)
    w = np.ascontiguousarray(np.asarray(W_qkv), dtype=np.float32)
    assert f1.shape == (B, N, D) and f2.shape == (B, N, D) and w.shape == (D, 3 * D)

    nc = _get_nc()
    in_maps = [{"feature1": f1[b], "feature2": f2[b], "w_qkv": w} for b in range(B)]
    try:
        res = run_bass_kernel_spmd(nc, in_maps, list(range(B))).results
    except Exception:
        res = run_bass_kernel_spmd(nc, in_maps, list(range(B))).results
    o1 = np.stack([res[b]["out1"] for b in range(B)]).astype(np.float32)
    o2 = np.stack([res[b]["out2"] for b in range(B)]).astype(np.float32)
    return o1, o2


if __name__ == "__main__":
    f1 = np.load("/root/problem/cache/f1.npy")
    f2 = np.load("/root/problem/cache/f2.npy")
    w = np.load("/root/problem/cache/W.npy")
    o1, o2 = kernel(f1, f2, w, 16)
    r1 = np.load("/root/problem/cache/r1.npy")
    r2 = np.load("/root/problem/cache/r2.npy")
    for nm, o, r in (("2to1", o1, r1), ("1to2", o2, r2)):
        err = np.abs(o - r).max()
        rel = err / np.abs(r).max()
        print(f"{nm}: absmax_err={err:.3e} rel={rel:.3e}")
